# revision 1
# baseline (speedup 1.0000x reference)
"""Tensor-parallel GQA attention prefill (B=1, T=2048, D=4096, 32 q-heads /
8 kv-heads) for 8 Trainium2 NeuronCores.

Sharding: head-parallel.  Core c owns q-heads [4c, 4c+4) and kv-head c.
  phase 1: Q/K/V projections in transposed layout (head-dim on partitions),
           RoPE applied via a rotation-matmul + two table multiplies.
  phase 2: per-head attention with scores held transposed (tk on
           partitions); softmax denominators come from a ones-matmul;
           fully-masked tiles are skipped (host inspects the mask tensor).
  phase 2.5: per-tq-chunk AllGather of attention slices, pipelined with
           the remaining attention chunks.
  phase 3: output projection column-shard per core, consuming each
           gathered chunk as it lands.

Matmul operands are bf16 (fp32 accumulation in PSUM); measured end-to-end
error vs the fp32 reference is ~4e-3 relative.

NOTE: faithful to the reference "bug" -- the q projection uses wo_w/wo_b.
"""

import numpy as np
import ml_dtypes

import bass_rust
import concourse.bass as bass
import concourse.mybir as mybir
import concourse.tile as tile
from concourse.bass_utils import run_bass_kernel_spmd
from concourse.masks import make_identity

# problem constants (self-contained; do not read spec.json)
DIM = 4096
NH = 32
NKV = 8
HD = 128
T = 2048
NCORE = 8
HPC = NH // NCORE      # 4 q heads per core
JPC = HPC * HD         # 512 output columns per core
P = 128
NT = T // 512          # 4 free-dim chunks of 512
NKC = DIM // P         # 32 contraction chunks in the projections
TKC = T // P           # 16 tk chunks in attention
SCALE = 1.0 / float(np.sqrt(HD))

F32 = mybir.dt.float32
BF16 = mybir.dt.bfloat16

# mask tile classification
MSK_SKIP, MSK_ZERO, MSK_ADD = 0, 1, 2


def legalize_waits(nc, max_waits=1):
    """Hoist excess on_wait conditions onto preceding nop instructions.

    This walrus build rejects instructions carrying more than a couple of
    sync-wait commands; engines execute their queue in order, so a nop that
    waits immediately before the real instruction is equivalent.
    """
    n_new = 0
    for f in nc.m.functions:
        for bb in f.blocks:
            insts = bb.instructions
            new = []
            for ins in list(insts):
                si = ins.sync_info
                waits = list(si.on_wait) if si is not None and si.on_wait else []
                if len(waits) > max_waits:
                    hoist = waits[:-max_waits]
                    keep = waits[-max_waits:]
                    for j in range(0, len(hoist), max_waits):
                        chunk = hoist[j:j + max_waits]
                        nop = mybir.InstNoOp(
                            name=f"{ins.name}_hw{j}",
                            engine=ins.engine,
                            sync_info=bass_rust.SyncInfo(
                                on_wait=chunk, on_update=[]),
                        )
                        new.append(nop)
                        n_new += 1
                    ins.sync_info = bass_rust.SyncInfo(
                        on_wait=keep,
                        on_update=list(si.on_update) if si.on_update else [])
                new.append(ins)
            insts.clear()
            insts.extend(new)
    return n_new


def _classify_mask(mask):
    """Per (tk-chunk, tq-chunk-of-512) classification of the additive mask.

    Returns (klass, col0) where col0[k, c] is the first tq column (multiple
    of 128) of the chunk that is not fully masked -- matmuls/exp for the
    columns before it are skipped (their softmax weights are exactly 0).
    """
    klass = np.empty((TKC, NT), dtype=np.int32)
    col0 = np.zeros((TKC, NT), dtype=np.int32)
    for k in range(TKC):
        for c in range(NT):
            blk = mask[c * 512:(c + 1) * 512, k * P:(k + 1) * P]
            mx = float(blk.max())
            mn = float(blk.min())
            if mx < -80.0:
                klass[k, c] = MSK_SKIP
                continue
            if mx == 0.0 and mn == 0.0:
                klass[k, c] = MSK_ZERO
            else:
                klass[k, c] = MSK_ADD
            # leading fully-masked tq columns, rounded down to 128
            colmax = blk.max(axis=1)          # per-tq-row max over this tile
            nz = np.nonzero(colmax >= -80.0)[0]
            first = int(nz[0]) if len(nz) else 0
            first = (first // P) * P
            # only safe to skip if every column before `first` is fully masked
            if first > 0 and float(blk[:first].max()) < -80.0:
                col0[k, c] = first
    # never allow a fully-empty (all-skip) tq chunk; keep one tile live
    for c in range(NT):
        if all(klass[k, c] == MSK_SKIP for k in range(TKC)):
            klass[min(c * 4, TKC - 1), c] = MSK_ADD
    return klass, col0


def _build_module(klass, col0, phases=(1, 2, 25, 3)):
    nc = bass.Bass()

    # inputs are pre-reblocked on the host so every DMA is contiguous
    xTb = nc.declare_dram_parameter("xTb", [NT, DIM, 512], BF16, isOutput=False)
    woT = nc.declare_dram_parameter("woT", [DIM, JPC], BF16, isOutput=False)
    wkT = nc.declare_dram_parameter("wkT", [DIM, HD], BF16, isOutput=False)
    wvT = nc.declare_dram_parameter("wvT", [DIM, HD], BF16, isOutput=False)
    qb = nc.declare_dram_parameter("qb", [P, HPC], F32, isOutput=False)
    kb = nc.declare_dram_parameter("kb", [P, 1], F32, isOutput=False)
    vb = nc.declare_dram_parameter("vb", [P, 1], F32, isOutput=False)
    maskTb = nc.declare_dram_parameter("maskTb", [NT, T, 512], BF16,
                                       isOutput=False)
    cost = nc.declare_dram_parameter("cost", [P, T], F32, isOutput=False)
    sint = nc.declare_dram_parameter("sint", [P, T], F32, isOutput=False)
    rT = nc.declare_dram_parameter("rT", [P, P], BF16, isOutput=False)
    outTb = nc.declare_dram_parameter("outTb", [NT, JPC, 512], F32,
                                      isOutput=True)

    ag_in = nc.dram_tensor("ag_in", [NT, JPC, 512], BF16)
    ag_out = nc.dram_tensor("ag_out", [NT, NCORE * JPC, 512], BF16,
                            addr_space="Shared")

    with tile.TileContext(nc) as tc:
        with (
            tc.tile_pool(name="wpool", bufs=1) as wpool,
            tc.tile_pool(name="const", bufs=1) as constp,
            tc.tile_pool(name="qkv", bufs=1) as qkvp,
            tc.tile_pool(name="xs", bufs=8) as xsp,
            tc.tile_pool(name="stage", bufs=4) as stagep,
            tc.tile_pool(name="att", bufs=4) as attp,
            tc.tile_pool(name="acc", bufs=4, space="PSUM") as accp,
            tc.tile_pool(name="satt", bufs=2, space="PSUM") as sattp,
            tc.tile_pool(name="attden", bufs=2, space="PSUM") as adp,
        ):
            # ---- resident weights / tables -------------------------------
            # wo split with tiny leading pieces so the first matmuls start
            # as soon as possible
            wo_s = wpool.tile([P, NKC, JPC], BF16)
            for klo, khi in ((0, 1), (1, 2), (2, 4), (4, 8), (8, 16), (16, 32)):
                nc.sync.dma_start(
                    out=wo_s[:, klo:khi, :],
                    in_=woT[klo * P:khi * P, :].rearrange("(k p) j -> p k j",
                                                          p=P))
            wk_s = wpool.tile([P, NKC, HD], BF16)
            nc.sync.dma_start(out=wk_s, in_=wkT[:, :].rearrange(
                "(k p) j -> p k j", p=P))
            wv_s = wpool.tile([P, NKC, HD], BF16)
            nc.sync.dma_start(out=wv_s, in_=wvT[:, :].rearrange(
                "(k p) j -> p k j", p=P))

            cos_s = constp.tile([P, T], F32)
            sin_s = constp.tile([P, T], F32)
            nc.sync.dma_start(out=cos_s, in_=cost[:, :])
            nc.sync.dma_start(out=sin_s, in_=sint[:, :])

            rT_s = constp.tile([P, P], BF16)
            nc.sync.dma_start(out=rT_s, in_=rT[:, :])
            qb_s = constp.tile([P, HPC], F32)
            kb_s = constp.tile([P, 1], F32)
            vb_s = constp.tile([P, 1], F32)
            nc.sync.dma_start(out=qb_s, in_=qb[:, :])
            nc.sync.dma_start(out=kb_s, in_=kb[:, :])
            nc.sync.dma_start(out=vb_s, in_=vb[:, :])

            ones_s = constp.tile([P, P], BF16)
            nc.vector.memset(ones_s, 1.0)
            ident_s = constp.tile([P, P], BF16)
            make_identity(nc, ident_s)

            # persistent Q/K/V in rope-d transposed layout
            qT_s = qkvp.tile([P, HPC, T], BF16)   # [hd, head, t]
            kT_s = qkvp.tile([P, T], BF16)        # [hd, t]
            vN_s = qkvp.tile([P, TKC, HD], BF16)  # [tk%128, tk//128, hd]

            # ---- phases 1+2 interleaved per t-chunk ----------------------
            # attention for chunk c only needs projections from chunks <= c,
            # so it is emitted right after chunk n=c's projections; the
            # scheduler fills its exp-latency bubbles with the next chunk's
            # projection matmuls, and the chunk's all-gather fires early.
            for n in range(NT):
                ts = slice(n * 512, (n + 1) * 512)
                if 1 in phases:
                    # x for this t-chunk: four 8-k-chunk quarter tiles (the
                    # very first quarter arrives in two halves)
                    xtq = []
                    for q in range(4):
                        ks = slice(q * 8 * P, (q + 1) * 8 * P)
                        xq = xsp.tile([P, 8, 512], BF16, name=f"xt{n}_{q}",
                                      tag="xs")
                        eng = nc.gpsimd
                        if n == 0 and q == 0:
                            for ha, hb in ((0, 2), (2, 8)):
                                eng.dma_start(
                                    out=xq[:, ha:hb, :],
                                    in_=xTb[n, ha * P:hb * P, :].rearrange(
                                        "(k p) t -> p k t", p=P))
                        else:
                            eng.dma_start(
                                out=xq,
                                in_=xTb[n, ks, :].rearrange("(k p) t -> p k t",
                                                            p=P))
                        xtq.append(xq)

                    def xt_sl(k):
                        return xtq[k // 8][:, k % 8, :]

                    acc_tiles = []
                    for m in range(HPC + 2):  # 4 q-head tiles, k, v
                        pacc = accp.tile([P, 512], F32, name=f"pacc{n}_{m}",
                                         tag="acc")
                        acc_tiles.append(pacc)
                    # q-pass
                    for k in range(NKC):
                        for m in range(HPC):
                            nc.tensor.matmul(
                                acc_tiles[m],
                                lhsT=wo_s[:, k, m * P:(m + 1) * P],
                                rhs=xt_sl(k),
                                start=(k == 0),
                                stop=(k == NKC - 1),
                            )
                    # rope for the 4 q tiles
                    for m in range(HPC):
                        braw = stagep.tile([P, 512], BF16, name=f"braw{n}_{m}",
                                           tag="braw")
                        nc.scalar.add(braw, acc_tiles[m], qb_s[:, m:m + 1])
                        rot_ps = accp.tile([P, 512], F32, name=f"rot{n}_{m}",
                                           tag="acc")
                        nc.tensor.matmul(rot_ps, lhsT=rT_s, rhs=braw,
                                         start=True, stop=True)
                        dst = qT_s[:, m, ts]
                        tmp = stagep.tile([P, 512], F32, name=f"tmp{n}_{m}",
                                          tag="stage")
                        nc.vector.tensor_mul(tmp, rot_ps, sin_s[:, ts])
                        nc.vector.tensor_mul(dst, braw, cos_s[:, ts])
                        nc.vector.tensor_add(dst, dst, tmp)
                    # kv-pass
                    for k in range(NKC):
                        nc.tensor.matmul(
                            acc_tiles[HPC], lhsT=wk_s[:, k, :], rhs=xt_sl(k),
                            start=(k == 0), stop=(k == NKC - 1),
                        )
                        nc.tensor.matmul(
                            acc_tiles[HPC + 1], lhsT=wv_s[:, k, :], rhs=xt_sl(k),
                            start=(k == 0), stop=(k == NKC - 1),
                        )
                    # rope for the k tile
                    braw = stagep.tile([P, 512], BF16, name=f"brawk{n}",
                                       tag="braw")
                    nc.scalar.add(braw, acc_tiles[HPC], kb_s[:, 0:1])
                    rot_ps = accp.tile([P, 512], F32, name=f"rotk{n}", tag="acc")
                    nc.tensor.matmul(rot_ps, lhsT=rT_s, rhs=braw,
                                     start=True, stop=True)
                    tmp = stagep.tile([P, 512], F32, name=f"tmpk{n}", tag="stage")
                    nc.vector.tensor_mul(tmp, rot_ps, sin_s[:, ts])
                    nc.vector.tensor_mul(kT_s[:, ts], braw, cos_s[:, ts])
                    nc.vector.tensor_add(kT_s[:, ts], kT_s[:, ts], tmp)

                    # v: bias then transpose into natural layout
                    v_st = stagep.tile([P, 512], BF16, name=f"vst{n}", tag="braw")
                    nc.scalar.add(v_st, acc_tiles[HPC + 1], vb_s[:, 0:1])
                    for j in range(4):
                        vt_ps = accp.tile([P, P], BF16, name=f"vt{n}_{j}",
                                          tag="acc")
                        nc.tensor.transpose(vt_ps, v_st[:, j * P:(j + 1) * P],
                                            ident_s)
                        nc.scalar.copy(vN_s[:, n * 4 + j, :], vt_ps)

                if 2 not in phases:
                    continue
                # ---- attention for chunk c = n + its all-gather ----------
                c = n
                cs = slice(c * 512, (c + 1) * 512)
                act_ks = [k for k in range(TKC) if klass[k, c] != MSK_SKIP]
                add_ks = [k for k in act_ks if klass[k, c] == MSK_ADD]
                mtiles = {}
                for k in add_ks:
                    mt = attp.tile([P, 512], BF16, name=f"mt{c}_{k}", tag="msk",
                                   bufs=max(2, len(add_ks) + 1))
                    nc.sync.dma_start(out=mt, in_=maskTb[c, k * P:(k + 1) * P, :])
                    mtiles[k] = mt
                for h in range(HPC):
                    attn_ps = adp.tile([P, 512], F32, name=f"apv{c}_{h}",
                                       tag="attden")
                    den_ps = adp.tile([P, 512], F32, name=f"den{c}_{h}",
                                      tag="attden")
                    nact = len(act_ks)
                    for i, k in enumerate(act_ks):
                        # leading fully-masked tq columns contribute exactly 0
                        # after exp, so shrink the tile.  The first matmul of
                        # each accumulation group stays full width so
                        # start=True clears the whole psum bank.
                        off = 0 if i == 0 else int(col0[k, c])
                        qs = slice(c * 512 + off, (c + 1) * 512)
                        s_ps = sattp.tile([P, 512], F32, name=f"sps{c}_{h}_{k}",
                                          tag="satt")
                        nc.tensor.matmul(
                            s_ps[:, off:],
                            lhsT=kT_s[:, k * P:(k + 1) * P],
                            rhs=qT_s[:, h, qs],
                            start=True, stop=True,
                        )
                        if k in mtiles:
                            nc.vector.tensor_add(s_ps[:, off:], s_ps[:, off:],
                                                 mtiles[k][:, off:])
                        e_sb = attp.tile([P, 512], BF16, name=f"e{c}_{h}_{k}",
                                         tag="exp", bufs=6)
                        # exp(SCALE * s + mask): mask was pre-divided by
                        # SCALE on the host, so the add can happen upstream.
                        nc.scalar.activation(
                            e_sb[:, off:], s_ps[:, off:],
                            mybir.ActivationFunctionType.Exp, scale=SCALE)
                        nc.tensor.matmul(
                            attn_ps[:, off:], lhsT=vN_s[:, k, :],
                            rhs=e_sb[:, off:],
                            start=(i == 0), stop=(i == nact - 1),
                        )
                        nc.tensor.matmul(
                            den_ps[:, off:], lhsT=ones_s, rhs=e_sb[:, off:],
                            start=(i == 0), stop=(i == nact - 1),
                        )
                    rcp = attp.tile([P, 512], F32, name=f"rcp{c}_{h}", tag="rcp",
                                    bufs=2)
                    nc.vector.reciprocal(rcp, den_ps)
                    attn_sb = stagep.tile([P, 512], BF16, name=f"ao{c}_{h}",
                                          tag="braw")
                    nc.vector.tensor_mul(attn_sb, attn_ps, rcp)
                    nc.sync.dma_start(out=ag_in[c, h * P:(h + 1) * P, :],
                                      in_=attn_sb)
                if 25 in phases:
                    nc.gpsimd.collective_compute(
                        "AllGather",
                        mybir.AluOpType.bypass,
                        replica_groups=[list(range(NCORE))],
                        ins=[ag_in[c]],
                        outs=[ag_out[c]],
                    )

            # ---- phase 3: output projection ------------------------------
            for n in range(NT) if 3 in phases else []:
                rq_tiles = []
                for q in range(4):
                    ks = slice(q * 8 * P, (q + 1) * 8 * P)
                    rq = xsp.tile([P, 8, 512], BF16, name=f"r{n}_{q}", tag="xs")
                    nc.gpsimd.dma_start(
                        out=rq,
                        in_=ag_out[n, ks, :].rearrange("(k p) t -> p k t", p=P))
                    rq_tiles.append(rq)

                def r_sl(k):
                    return rq_tiles[k // 8][:, k % 8, :]

                o_acc = []
                for m in range(HPC):
                    po = accp.tile([P, 512], F32, name=f"oacc{n}_{m}", tag="acc")
                    o_acc.append(po)
                for k in range(NKC):
                    for m in range(HPC):
                        nc.tensor.matmul(
                            o_acc[m],
                            lhsT=wo_s[:, k, m * P:(m + 1) * P],
                            rhs=r_sl(k),
                            start=(k == 0),
                            stop=(k == NKC - 1),
                        )
                for m in range(HPC):
                    o_sb = stagep.tile([P, 512], F32, name=f"o{n}_{m}",
                                       tag="stage")
                    nc.scalar.add(o_sb, o_acc[m], qb_s[:, m:m + 1])
                    nc.sync.dma_start(out=outTb[n, m * P:(m + 1) * P, :],
                                      in_=o_sb)

    legalize_waits(nc)
    return nc


def _marshal_inputs(x, freqs_cos, freqs_sin, mask, wk_w, wk_b, wv_w, wv_b,
                    wo_w, wo_b):
    bf = ml_dtypes.bfloat16
    x = np.asarray(x, np.float32)
    mask = np.asarray(mask, np.float32)
    cos = np.asarray(freqs_cos, np.float32)
    sin = np.asarray(freqs_sin, np.float32)
    wk_w = np.asarray(wk_w, np.float32)
    wv_w = np.asarray(wv_w, np.float32)
    wo_w = np.asarray(wo_w, np.float32)
    wk_b = np.asarray(wk_b, np.float32)
    wv_b = np.asarray(wv_b, np.float32)
    wo_b = np.asarray(wo_b, np.float32)

    xT = x.reshape(T, DIM).T                       # (DIM, T)
    xTb = np.ascontiguousarray(
        xT.reshape(DIM, NT, 512).transpose(1, 0, 2).astype(bf))
    # mask applied on-device as exp(SCALE*s + SCALE*maskT): pre-divide, and
    # reblock (tq-chunk, tk, tq') so every mask tile DMA is contiguous
    maskT = mask.T / np.float32(SCALE)             # (tk, tq)
    maskTb = np.ascontiguousarray(
        maskT.reshape(T, NT, 512).transpose(1, 0, 2).astype(bf))

    cos2 = np.repeat(cos.T, 2, axis=0)  # (128, T): rows 2i,2i+1 = cos[:, i]
    sin2 = np.repeat(sin.T, 2, axis=0)

    # rotation matmul constant: out = R @ q with rot[2i] = -q[2i+1],
    # rot[2i+1] = q[2i]; lhsT layout (R transposed).
    RT = np.zeros((P, P), np.float32)
    idx = np.arange(0, P, 2)
    RT[idx + 1, idx] = -1.0
    RT[idx, idx + 1] = 1.0

    common = dict(
        xTb=xTb, maskTb=maskTb,
        cost=np.ascontiguousarray(cos2),
        sint=np.ascontiguousarray(sin2),
        rT=RT.astype(bf),
    )

    in_maps = []
    for cix in range(NCORE):
        jlo = cix * JPC
        klo = cix * HD
        m = dict(common)
        m["woT"] = np.ascontiguousarray(wo_w[jlo:jlo + JPC, :].T.astype(bf))
        m["wkT"] = np.ascontiguousarray(wk_w[klo:klo + HD, :].T.astype(bf))
        m["wvT"] = np.ascontiguousarray(wv_w[klo:klo + HD, :].T.astype(bf))
        m["qb"] = np.ascontiguousarray(wo_b[jlo:jlo + JPC].reshape(HPC, P).T)
        m["kb"] = np.ascontiguousarray(wk_b[klo:klo + HD].reshape(1, P).T)
        m["vb"] = np.ascontiguousarray(wv_b[klo:klo + HD].reshape(1, P).T)
        in_maps.append(m)
    return in_maps, mask


def run(inputs, trace=False):
    """Build, run on 8 cores, return (full_output, BassKernelResults)."""
    in_maps, mask = _marshal_inputs(
        inputs["x"], inputs["freqs_cos"], inputs["freqs_sin"], inputs["mask"],
        inputs["wk_w"], inputs["wk_b"], inputs["wv_w"], inputs["wv_b"],
        inputs["wo_w"], inputs["wo_b"])
    klass, col0 = _classify_mask(mask)
    nc = _build_module(klass, col0)
    res = run_bass_kernel_spmd(nc, in_maps, core_ids=list(range(NCORE)),
                               trace=trace)
    out = np.empty((DIM, T), np.float32)
    for cix in range(NCORE):
        ob = res.results[cix]["outTb"]          # (NT, JPC, 512)
        for n in range(NT):
            out[cix * JPC:(cix + 1) * JPC, n * 512:(n + 1) * 512] = ob[n]
    out = out.T  # (T, DIM)
    return np.ascontiguousarray(out[None, :, :]).astype(np.float32), res


def kernel(**inputs):
    out, _ = run(inputs, trace=False)
    return out



# revision 3
# speedup vs baseline: 1.2365x; 1.2365x over previous
"""Tensor-parallel GQA attention prefill (B=1, T=2048, D=4096, 32 q-heads /
8 kv-heads) for 8 Trainium2 NeuronCores.

Sharding: head-parallel.  Core c owns q-heads [4c, 4c+4) and kv-head c.
  phase 1: Q/K/V projections in transposed layout (head-dim on partitions),
           RoPE applied via a rotation-matmul + two table multiplies.
  phase 2: per-head attention with scores held transposed (tk on
           partitions); softmax denominators come from a ones-matmul;
           fully-masked tiles are skipped (host inspects the mask tensor).
  phase 3: output projection sharded over the CONTRACTION dim: each core
           multiplies its local attention slice (512 rows) against its
           512-row slice of wo, producing partial sums for ALL 4096 output
           dims; a per-tq-chunk ReduceScatter (bf16) then both sums the
           partials and hands each core its own 512 output rows.
  phase 4: bias add + writeback of the scattered result.

Matmul operands are bf16 (fp32 accumulation in PSUM); measured end-to-end
error vs the fp32 reference is ~4e-3 relative.

NOTE: faithful to the reference "bug" -- the q projection uses wo_w/wo_b.
"""

import numpy as np
import ml_dtypes

import bass_rust
import concourse.bass as bass
import concourse.mybir as mybir
import concourse.tile as tile
from concourse.bass_utils import run_bass_kernel_spmd
from concourse.masks import make_identity

# problem constants (self-contained; do not read spec.json)
DIM = 4096
NH = 32
NKV = 8
HD = 128
T = 2048
NCORE = 8
HPC = NH // NCORE      # 4 q heads per core
JPC = HPC * HD         # 512 output columns per core
P = 128
NT = T // 512          # 4 free-dim chunks of 512
NKC = DIM // P         # 32 contraction chunks in the projections
TKC = T // P           # 16 tk chunks in attention
SCALE = 1.0 / float(np.sqrt(HD))

F32 = mybir.dt.float32
BF16 = mybir.dt.bfloat16

# mask tile classification
MSK_SKIP, MSK_ZERO, MSK_ADD = 0, 1, 2


def legalize_waits(nc, max_waits=1):
    """Hoist excess on_wait conditions onto preceding nop instructions.

    This walrus build rejects instructions carrying more than a couple of
    sync-wait commands; engines execute their queue in order, so a nop that
    waits immediately before the real instruction is equivalent.
    """
    n_new = 0
    for f in nc.m.functions:
        for bb in f.blocks:
            insts = bb.instructions
            new = []
            for ins in list(insts):
                si = ins.sync_info
                waits = list(si.on_wait) if si is not None and si.on_wait else []
                if len(waits) > max_waits:
                    hoist = waits[:-max_waits]
                    keep = waits[-max_waits:]
                    for j in range(0, len(hoist), max_waits):
                        chunk = hoist[j:j + max_waits]
                        nop = mybir.InstNoOp(
                            name=f"{ins.name}_hw{j}",
                            engine=ins.engine,
                            sync_info=bass_rust.SyncInfo(
                                on_wait=chunk, on_update=[]),
                        )
                        new.append(nop)
                        n_new += 1
                    ins.sync_info = bass_rust.SyncInfo(
                        on_wait=keep,
                        on_update=list(si.on_update) if si.on_update else [])
                new.append(ins)
            insts.clear()
            insts.extend(new)
    return n_new


def _classify_mask(mask):
    """Per (tk-chunk, tq-chunk-of-512) classification of the additive mask.

    Returns (klass, col0) where col0[k, c] is the first tq column (multiple
    of 128) of the chunk that is not fully masked -- matmuls/exp for the
    columns before it are skipped (their softmax weights are exactly 0).
    """
    klass = np.empty((TKC, NT), dtype=np.int32)
    col0 = np.zeros((TKC, NT), dtype=np.int32)
    for k in range(TKC):
        for c in range(NT):
            blk = mask[c * 512:(c + 1) * 512, k * P:(k + 1) * P]
            mx = float(blk.max())
            mn = float(blk.min())
            if mx < -80.0:
                klass[k, c] = MSK_SKIP
                continue
            if mx == 0.0 and mn == 0.0:
                klass[k, c] = MSK_ZERO
            else:
                klass[k, c] = MSK_ADD
            # leading fully-masked tq columns, rounded down to 128
            colmax = blk.max(axis=1)          # per-tq-row max over this tile
            nz = np.nonzero(colmax >= -80.0)[0]
            first = int(nz[0]) if len(nz) else 0
            first = (first // P) * P
            # only safe to skip if every column before `first` is fully masked
            if first > 0 and float(blk[:first].max()) < -80.0:
                col0[k, c] = first
    # never allow a fully-empty (all-skip) tq chunk; keep one tile live
    for c in range(NT):
        if all(klass[k, c] == MSK_SKIP for k in range(TKC)):
            klass[min(c * 4, TKC - 1), c] = MSK_ADD
    return klass, col0


def _build_module(klass, col0, phases=(1, 2, 25, 3)):
    nc = bass.Bass()

    # inputs are pre-reblocked on the host so every DMA is contiguous
    xTb = nc.declare_dram_parameter("xTb", [NT, DIM, 512], BF16, isOutput=False)
    woT = nc.declare_dram_parameter("woT", [DIM, JPC], BF16, isOutput=False)
    woR = nc.declare_dram_parameter("woR", [JPC, DIM], BF16, isOutput=False)
    wkT = nc.declare_dram_parameter("wkT", [DIM, HD], BF16, isOutput=False)
    wvT = nc.declare_dram_parameter("wvT", [DIM, HD], BF16, isOutput=False)
    qb = nc.declare_dram_parameter("qb", [P, HPC], F32, isOutput=False)
    kb = nc.declare_dram_parameter("kb", [P, 1], F32, isOutput=False)
    vb = nc.declare_dram_parameter("vb", [P, 1], F32, isOutput=False)
    maskTb = nc.declare_dram_parameter("maskTb", [NT, T, 512], BF16,
                                       isOutput=False)
    cost = nc.declare_dram_parameter("cost", [P, T], BF16, isOutput=False)
    sint = nc.declare_dram_parameter("sint", [P, T], BF16, isOutput=False)
    rT = nc.declare_dram_parameter("rT", [P, P], BF16, isOutput=False)
    outTb = nc.declare_dram_parameter("outTb", [NT, JPC, 512], F32,
                                      isOutput=True)

    rs_in = nc.dram_tensor("rs_in", [NT, NCORE * JPC, 512], BF16)
    rs_out = nc.dram_tensor("rs_out", [NT, JPC, 512], BF16)

    with tile.TileContext(nc) as tc:
        with (
            tc.tile_pool(name="wpool", bufs=1) as wpool,
            tc.tile_pool(name="const", bufs=1) as constp,
            tc.tile_pool(name="qkv", bufs=1) as qkvp,
            tc.tile_pool(name="qc", bufs=2) as qcp,
            tc.tile_pool(name="aout", bufs=2) as aop,
            tc.tile_pool(name="ro", bufs=2) as rop,
            tc.tile_pool(name="xs", bufs=5) as xsp,
            tc.tile_pool(name="stage", bufs=4) as stagep,
            tc.tile_pool(name="att", bufs=4) as attp,
            tc.tile_pool(name="acc", bufs=4, space="PSUM") as accp,
            tc.tile_pool(name="satt", bufs=2, space="PSUM") as sattp,
            tc.tile_pool(name="attden", bufs=2, space="PSUM") as adp,
        ):
            # ---- resident weights / tables -------------------------------
            # wo split with tiny leading pieces so the first matmuls start
            # as soon as possible
            wo_s = wpool.tile([P, NKC, JPC], BF16)
            for klo, khi in ((0, 1), (1, 2), (2, 4), (4, 8), (8, 16), (16, 32)):
                nc.sync.dma_start(
                    out=wo_s[:, klo:khi, :],
                    in_=woT[klo * P:khi * P, :].rearrange("(k p) j -> p k j",
                                                          p=P))
            wk_s = wpool.tile([P, NKC, HD], BF16)
            nc.sync.dma_start(out=wk_s, in_=wkT[:, :].rearrange(
                "(k p) j -> p k j", p=P))
            wv_s = wpool.tile([P, NKC, HD], BF16)
            nc.sync.dma_start(out=wv_s, in_=wvT[:, :].rearrange(
                "(k p) j -> p k j", p=P))

            cos_s = constp.tile([P, T], BF16)
            sin_s = constp.tile([P, T], BF16)
            nc.sync.dma_start(out=cos_s, in_=cost[:, :])
            nc.sync.dma_start(out=sin_s, in_=sint[:, :])

            rT_s = constp.tile([P, P], BF16)
            nc.sync.dma_start(out=rT_s, in_=rT[:, :])
            qb_s = constp.tile([P, HPC], F32)
            kb_s = constp.tile([P, 1], F32)
            vb_s = constp.tile([P, 1], F32)
            nc.sync.dma_start(out=qb_s, in_=qb[:, :])
            nc.sync.dma_start(out=kb_s, in_=kb[:, :])
            nc.sync.dma_start(out=vb_s, in_=vb[:, :])

            # row-slice of wo for the contraction-sharded output projection
            wo3_s = wpool.tile([P, HPC, DIM], BF16)
            nc.sync.dma_start(out=wo3_s, in_=woR[:, :].rearrange(
                "(m p) o -> p m o", p=P))

            ones_s = constp.tile([P, P], BF16)
            nc.vector.memset(ones_s, 1.0)
            ident_s = constp.tile([P, P], BF16)
            make_identity(nc, ident_s)

            # persistent K/V in rope-d transposed layout (Q is per-chunk)
            kT_s = qkvp.tile([P, T], BF16)        # [hd, t]
            vN_s = qkvp.tile([P, TKC, HD], BF16)  # [tk%128, tk//128, hd]

            # ---- phases interleaved per t-chunk --------------------------
            # attention for chunk c only needs projections from chunks <= c,
            # so it is emitted right after chunk n=c's projections; its
            # partial output projection and ReduceScatter follow immediately
            # and overlap the next chunk's projections.
            for n in range(NT):
                ts = slice(n * 512, (n + 1) * 512)
                # x for this t-chunk: four 8-k-chunk quarter tiles (the
                # very first quarter arrives in two halves).  Issued on the
                # Pool queue BEFORE the previous chunk's psum-copy burst so
                # the data is in flight during that chunk's output proj.
                xtq = []
                for q in range(4):
                    ks = slice(q * 8 * P, (q + 1) * 8 * P)
                    xq = xsp.tile([P, 8, 512], BF16, name=f"xt{n}_{q}",
                                  tag="xs")
                    eng = nc.gpsimd
                    if n == 0 and q == 0:
                        for ha, hb in ((0, 2), (2, 8)):
                            eng.dma_start(
                                out=xq[:, ha:hb, :],
                                in_=xTb[n, ha * P:hb * P, :].rearrange(
                                    "(k p) t -> p k t", p=P))
                    else:
                        eng.dma_start(
                            out=xq,
                            in_=xTb[n, ks, :].rearrange("(k p) t -> p k t",
                                                        p=P))
                    xtq.append(xq)

                def xt_sl(k):
                    return xtq[k // 8][:, k % 8, :]

                acc_tiles = []
                for m in range(HPC + 2):  # 4 q-head tiles, k, v
                    pacc = accp.tile([P, 512], F32, name=f"pacc{n}_{m}",
                                     tag="acc")
                    acc_tiles.append(pacc)
                # q-pass
                for k in range(NKC):
                    for m in range(HPC):
                        nc.tensor.matmul(
                            acc_tiles[m],
                            lhsT=wo_s[:, k, m * P:(m + 1) * P],
                            rhs=xt_sl(k),
                            start=(k == 0),
                            stop=(k == NKC - 1),
                        )
                # q biases on Act while the kv-pass runs on PE
                braw_q = []
                for m in range(HPC):
                    braw = stagep.tile([P, 512], BF16, name=f"braw{n}_{m}",
                                       tag="braw")
                    nc.scalar.add(braw, acc_tiles[m], qb_s[:, m:m + 1])
                    braw_q.append(braw)
                # kv-pass
                for k in range(NKC):
                    nc.tensor.matmul(
                        acc_tiles[HPC], lhsT=wk_s[:, k, :], rhs=xt_sl(k),
                        start=(k == 0), stop=(k == NKC - 1),
                    )
                    nc.tensor.matmul(
                        acc_tiles[HPC + 1], lhsT=wv_s[:, k, :], rhs=xt_sl(k),
                        start=(k == 0), stop=(k == NKC - 1),
                    )
                # v bias first (its transpose is the earliest PE consumer
                # after the rotation matmuls), then k bias
                v_st = stagep.tile([P, 512], BF16, name=f"vst{n}", tag="braw")
                nc.scalar.add(v_st, acc_tiles[HPC + 1], vb_s[:, 0:1])
                brawk = stagep.tile([P, 512], BF16, name=f"brawk{n}",
                                    tag="braw")
                nc.scalar.add(brawk, acc_tiles[HPC], kb_s[:, 0:1])

                # rotation matmuls for q tiles + k tile (PE, after kv-pass)
                qc_s = qcp.tile([P, HPC, 512], BF16, name=f"qc{n}", tag="qc")
                rot_q = []
                for m in range(HPC):
                    rot_ps = accp.tile([P, 512], F32, name=f"rot{n}_{m}",
                                       tag="acc")
                    nc.tensor.matmul(rot_ps, lhsT=rT_s, rhs=braw_q[m],
                                     start=True, stop=True)
                    rot_q.append(rot_ps)
                rot_k = accp.tile([P, 512], F32, name=f"rotk{n}", tag="acc")
                nc.tensor.matmul(rot_k, lhsT=rT_s, rhs=brawk,
                                 start=True, stop=True)
                # v transpose into natural layout
                for j in range(4):
                    vt_ps = accp.tile([P, P], BF16, name=f"vt{n}_{j}",
                                      tag="acc")
                    nc.tensor.transpose(vt_ps, v_st[:, j * P:(j + 1) * P],
                                        ident_s)
                    nc.scalar.copy(vN_s[:, n * 4 + j, :], vt_ps)

                # rope combine on DVE (all-bf16 for 2x mode where possible)
                for m in range(HPC):
                    dst = qc_s[:, m, :]
                    tmp = stagep.tile([P, 512], BF16, name=f"tmp{n}_{m}",
                                      tag="stage")
                    nc.vector.tensor_mul(tmp, rot_q[m], sin_s[:, ts])
                    nc.vector.tensor_mul(dst, braw_q[m], cos_s[:, ts])
                    nc.vector.tensor_add(dst, dst, tmp)
                tmpk = stagep.tile([P, 512], BF16, name=f"tmpk{n}", tag="stage")
                nc.vector.tensor_mul(tmpk, rot_k, sin_s[:, ts])
                nc.vector.tensor_mul(kT_s[:, ts], brawk, cos_s[:, ts])
                nc.vector.tensor_add(kT_s[:, ts], kT_s[:, ts], tmpk)

                # ---- attention for chunk c = n ---------------------------
                c = n
                act_ks = [k for k in range(TKC) if klass[k, c] != MSK_SKIP]
                add_ks = [k for k in act_ks if klass[k, c] == MSK_ADD]
                mtiles = {}
                for k in add_ks:
                    mt = attp.tile([P, 512], BF16, name=f"mt{c}_{k}", tag="msk",
                                   bufs=max(2, len(add_ks) + 1))
                    nc.sync.dma_start(out=mt, in_=maskTb[c, k * P:(k + 1) * P, :])
                    mtiles[k] = mt

                attn_c = aop.tile([P, HPC, 512], BF16, name=f"ac{c}", tag="ac")
                nact = len(act_ks)
                pairs = [(h, i, k) for h in range(HPC)
                         for i, k in enumerate(act_ks)]
                state = {}

                def emit_score(h, i, k):
                    off = 0 if i == 0 else int(col0[k, c])
                    s_ps = sattp.tile([P, 512], F32, name=f"sps{c}_{h}_{k}",
                                      tag="satt")
                    nc.tensor.matmul(
                        s_ps[:, off:],
                        lhsT=kT_s[:, k * P:(k + 1) * P],
                        rhs=qc_s[:, h, off:],
                        start=True, stop=True,
                    )
                    if k in mtiles:
                        nc.vector.tensor_add(s_ps[:, off:], s_ps[:, off:],
                                             mtiles[k][:, off:])
                    e_sb = attp.tile([P, 512], BF16, name=f"e{c}_{h}_{k}",
                                     tag="exp", bufs=6)
                    # exp(SCALE * s + mask): mask was pre-divided by
                    # SCALE on the host, so the add can happen upstream.
                    nc.scalar.activation(
                        e_sb[:, off:], s_ps[:, off:],
                        mybir.ActivationFunctionType.Exp, scale=SCALE)
                    state[(h, i)] = (e_sb, off)

                def emit_avden(h, i):
                    if i == 0:
                        state[h, "apv"] = adp.tile(
                            [P, 512], F32, name=f"apv{c}_{h}", tag="attden")
                        state[h, "den"] = adp.tile(
                            [P, 512], F32, name=f"den{c}_{h}", tag="attden")
                    e_sb, off = state.pop((h, i))
                    k = act_ks[i]
                    nc.tensor.matmul(
                        state[h, "apv"][:, off:], lhsT=vN_s[:, k, :],
                        rhs=e_sb[:, off:],
                        start=(i == 0), stop=(i == nact - 1),
                    )
                    nc.tensor.matmul(
                        state[h, "den"][:, off:], lhsT=ones_s,
                        rhs=e_sb[:, off:],
                        start=(i == 0), stop=(i == nact - 1),
                    )
                    if i == nact - 1:
                        rcp = attp.tile([P, 512], F32, name=f"rcp{c}_{h}",
                                        tag="rcp", bufs=2)
                        nc.vector.reciprocal(rcp, state.pop((h, "den")))
                        nc.vector.tensor_mul(attn_c[:, h, :],
                                             state.pop((h, "apv")), rcp)

                # software-pipelined emission: the score for pair j+1 sits
                # between pair j's score and its exp-dependent matmuls, so
                # the PE never stalls the full exp latency.
                for j, (h, i, k) in enumerate(pairs):
                    emit_score(h, i, k)
                    if j >= 1:
                        ph, pi, _ = pairs[j - 1]
                        emit_avden(ph, pi)
                emit_avden(*pairs[-1][:2])

                # ---- phase 3: partial output projection + ReduceScatter --
                # contraction over this core's 512 attention dims, all 4096
                # output dims; psum banks borrowed from the (idle) attention
                # pools so the next chunk's projections keep their own.
                for jo in range(NKC):
                    po = (sattp if jo % 2 == 0 else adp).tile(
                        [P, 512], F32, name=f"po{n}_{jo}",
                        tag="satt" if jo % 2 == 0 else "attden")
                    for m in range(HPC):
                        nc.tensor.matmul(
                            po,
                            lhsT=wo3_s[:, m, jo * P:(jo + 1) * P],
                            rhs=attn_c[:, m, :],
                            start=(m == 0),
                            stop=(m == HPC - 1),
                        )
                    osb = attp.tile([P, 512], BF16, name=f"osb{n}_{jo}",
                                    tag="osb", bufs=4)
                    nc.gpsimd.tensor_copy(osb, po)
                    nc.sync.dma_start(out=rs_in[n, jo * P:(jo + 1) * P, :],
                                      in_=osb)
                nc.gpsimd.collective_compute(
                    "ReduceScatter",
                    mybir.AluOpType.add,
                    replica_groups=[list(range(NCORE))],
                    ins=[rs_in[n]],
                    outs=[rs_out[n]],
                )

                # ---- phase 4: bias + writeback (SP + DVE) ----------------
                ro_s = rop.tile([P, HPC, 512], BF16, name=f"ro{n}", tag="ro")
                nc.sync.dma_start(out=ro_s, in_=rs_out[n].rearrange(
                    "(m p) t -> p m t", p=P))
                for m in range(HPC):
                    o_sb = stagep.tile([P, 512], F32, name=f"o{n}_{m}",
                                       tag="ostage", bufs=2)
                    nc.vector.tensor_scalar_add(o_sb, ro_s[:, m, :],
                                                qb_s[:, m:m + 1])
                    nc.sync.dma_start(out=outTb[n, m * P:(m + 1) * P, :],
                                      in_=o_sb)

    legalize_waits(nc)
    return nc


def _marshal_inputs(x, freqs_cos, freqs_sin, mask, wk_w, wk_b, wv_w, wv_b,
                    wo_w, wo_b):
    bf = ml_dtypes.bfloat16
    x = np.asarray(x, np.float32)
    mask = np.asarray(mask, np.float32)
    cos = np.asarray(freqs_cos, np.float32)
    sin = np.asarray(freqs_sin, np.float32)
    wk_w = np.asarray(wk_w, np.float32)
    wv_w = np.asarray(wv_w, np.float32)
    wo_w = np.asarray(wo_w, np.float32)
    wk_b = np.asarray(wk_b, np.float32)
    wv_b = np.asarray(wv_b, np.float32)
    wo_b = np.asarray(wo_b, np.float32)

    xT = x.reshape(T, DIM).T                       # (DIM, T)
    xTb = np.ascontiguousarray(
        xT.reshape(DIM, NT, 512).transpose(1, 0, 2).astype(bf))
    # mask applied on-device as exp(SCALE*s + SCALE*maskT): pre-divide, and
    # reblock (tq-chunk, tk, tq') so every mask tile DMA is contiguous
    maskT = mask.T / np.float32(SCALE)             # (tk, tq)
    maskTb = np.ascontiguousarray(
        maskT.reshape(T, NT, 512).transpose(1, 0, 2).astype(bf))

    cos2 = np.repeat(cos.T, 2, axis=0)  # (128, T): rows 2i,2i+1 = cos[:, i]
    sin2 = np.repeat(sin.T, 2, axis=0)

    # rotation matmul constant: out = R @ q with rot[2i] = -q[2i+1],
    # rot[2i+1] = q[2i]; lhsT layout (R transposed).
    RT = np.zeros((P, P), np.float32)
    idx = np.arange(0, P, 2)
    RT[idx + 1, idx] = -1.0
    RT[idx, idx + 1] = 1.0

    common = dict(
        xTb=xTb, maskTb=maskTb,
        cost=np.ascontiguousarray(cos2.astype(bf)),
        sint=np.ascontiguousarray(sin2.astype(bf)),
        rT=RT.astype(bf),
    )

    woT_full = wo_w.T  # (DIM in, DIM out): woT_full[d, o] = wo_w[o, d]
    in_maps = []
    for cix in range(NCORE):
        jlo = cix * JPC
        klo = cix * HD
        m = dict(common)
        m["woT"] = np.ascontiguousarray(wo_w[jlo:jlo + JPC, :].T.astype(bf))
        m["woR"] = np.ascontiguousarray(woT_full[jlo:jlo + JPC, :].astype(bf))
        m["wkT"] = np.ascontiguousarray(wk_w[klo:klo + HD, :].T.astype(bf))
        m["wvT"] = np.ascontiguousarray(wv_w[klo:klo + HD, :].T.astype(bf))
        m["qb"] = np.ascontiguousarray(wo_b[jlo:jlo + JPC].reshape(HPC, P).T)
        m["kb"] = np.ascontiguousarray(wk_b[klo:klo + HD].reshape(1, P).T)
        m["vb"] = np.ascontiguousarray(wv_b[klo:klo + HD].reshape(1, P).T)
        in_maps.append(m)
    return in_maps, mask


def run(inputs, trace=False):
    """Build, run on 8 cores, return (full_output, BassKernelResults)."""
    in_maps, mask = _marshal_inputs(
        inputs["x"], inputs["freqs_cos"], inputs["freqs_sin"], inputs["mask"],
        inputs["wk_w"], inputs["wk_b"], inputs["wv_w"], inputs["wv_b"],
        inputs["wo_w"], inputs["wo_b"])
    klass, col0 = _classify_mask(mask)
    nc = _build_module(klass, col0)
    res = run_bass_kernel_spmd(nc, in_maps, core_ids=list(range(NCORE)),
                               trace=trace)
    out = np.empty((DIM, T), np.float32)
    for cix in range(NCORE):
        ob = res.results[cix]["outTb"]          # (NT, JPC, 512)
        for n in range(NT):
            out[cix * JPC:(cix + 1) * JPC, n * 512:(n + 1) * 512] = ob[n]
    out = out.T  # (T, DIM)
    return np.ascontiguousarray(out[None, :, :]).astype(np.float32), res


def kernel(**inputs):
    out, _ = run(inputs, trace=False)
    return out


# revision 4
# speedup vs baseline: 1.2413x; 1.0039x over previous
"""Tensor-parallel GQA attention prefill (B=1, T=2048, D=4096, 32 q-heads /
8 kv-heads) for 8 Trainium2 NeuronCores.

Sharding: head-parallel.  Core c owns q-heads [4c, 4c+4) and kv-head c.
  phase 1: Q/K/V projections in transposed layout (head-dim on partitions),
           RoPE applied via a rotation-matmul + two table multiplies.
  phase 2: per-head attention with scores held transposed (tk on
           partitions); softmax denominators come from a ones-matmul;
           fully-masked tiles are skipped (host inspects the mask tensor).
  phase 3: output projection sharded over the CONTRACTION dim: each core
           multiplies its local attention slice (512 rows) against its
           512-row slice of wo, producing partial sums for ALL 4096 output
           dims; a per-tq-chunk ReduceScatter (bf16) then both sums the
           partials and hands each core its own 512 output rows.
  phase 4: bias add + writeback of the scattered result.

Matmul operands are bf16 (fp32 accumulation in PSUM); measured end-to-end
error vs the fp32 reference is ~4e-3 relative.

NOTE: faithful to the reference "bug" -- the q projection uses wo_w/wo_b.
"""

import numpy as np
import ml_dtypes

import bass_rust
import concourse.bass as bass
import concourse.mybir as mybir
import concourse.tile as tile
from concourse.bass_utils import run_bass_kernel_spmd
from concourse.masks import make_identity

# problem constants (self-contained; do not read spec.json)
DIM = 4096
NH = 32
NKV = 8
HD = 128
T = 2048
NCORE = 8
HPC = NH // NCORE      # 4 q heads per core
JPC = HPC * HD         # 512 output columns per core
P = 128
NT = T // 512          # 4 free-dim chunks of 512
NKC = DIM // P         # 32 contraction chunks in the projections
TKC = T // P           # 16 tk chunks in attention
SCALE = 1.0 / float(np.sqrt(HD))

F32 = mybir.dt.float32
BF16 = mybir.dt.bfloat16

# mask tile classification
MSK_SKIP, MSK_ZERO, MSK_ADD = 0, 1, 2


def legalize_waits(nc, max_waits=1):
    """Hoist excess on_wait conditions onto preceding nop instructions.

    This walrus build rejects instructions carrying more than a couple of
    sync-wait commands; engines execute their queue in order, so a nop that
    waits immediately before the real instruction is equivalent.
    """
    n_new = 0
    for f in nc.m.functions:
        for bb in f.blocks:
            insts = bb.instructions
            new = []
            for ins in list(insts):
                si = ins.sync_info
                waits = list(si.on_wait) if si is not None and si.on_wait else []
                if len(waits) > max_waits:
                    hoist = waits[:-max_waits]
                    keep = waits[-max_waits:]
                    for j in range(0, len(hoist), max_waits):
                        chunk = hoist[j:j + max_waits]
                        nop = mybir.InstNoOp(
                            name=f"{ins.name}_hw{j}",
                            engine=ins.engine,
                            sync_info=bass_rust.SyncInfo(
                                on_wait=chunk, on_update=[]),
                        )
                        new.append(nop)
                        n_new += 1
                    ins.sync_info = bass_rust.SyncInfo(
                        on_wait=keep,
                        on_update=list(si.on_update) if si.on_update else [])
                new.append(ins)
            insts.clear()
            insts.extend(new)
    return n_new


def _classify_mask(mask):
    """Per (tk-chunk, tq-chunk-of-512) classification of the additive mask.

    Returns (klass, col0) where col0[k, c] is the first tq column (multiple
    of 128) of the chunk that is not fully masked -- matmuls/exp for the
    columns before it are skipped (their softmax weights are exactly 0).
    """
    klass = np.empty((TKC, NT), dtype=np.int32)
    col0 = np.zeros((TKC, NT), dtype=np.int32)
    for k in range(TKC):
        for c in range(NT):
            blk = mask[c * 512:(c + 1) * 512, k * P:(k + 1) * P]
            mx = float(blk.max())
            mn = float(blk.min())
            if mx < -80.0:
                klass[k, c] = MSK_SKIP
                continue
            if mx == 0.0 and mn == 0.0:
                klass[k, c] = MSK_ZERO
            else:
                klass[k, c] = MSK_ADD
            # leading fully-masked tq columns, rounded down to 128
            colmax = blk.max(axis=1)          # per-tq-row max over this tile
            nz = np.nonzero(colmax >= -80.0)[0]
            first = int(nz[0]) if len(nz) else 0
            first = (first // P) * P
            # only safe to skip if every column before `first` is fully masked
            if first > 0 and float(blk[:first].max()) < -80.0:
                col0[k, c] = first
    # never allow a fully-empty (all-skip) tq chunk; keep one tile live
    for c in range(NT):
        if all(klass[k, c] == MSK_SKIP for k in range(TKC)):
            klass[min(c * 4, TKC - 1), c] = MSK_ADD
    return klass, col0


def _build_module(klass, col0, phases=(1, 2, 25, 3)):
    nc = bass.Bass()

    # inputs are pre-reblocked on the host so every DMA is contiguous
    xTb = nc.declare_dram_parameter("xTb", [NT, DIM, 512], BF16, isOutput=False)
    woT = nc.declare_dram_parameter("woT", [DIM, JPC], BF16, isOutput=False)
    woR = nc.declare_dram_parameter("woR", [JPC, DIM], BF16, isOutput=False)
    wkT = nc.declare_dram_parameter("wkT", [DIM, HD], BF16, isOutput=False)
    wvT = nc.declare_dram_parameter("wvT", [DIM, HD], BF16, isOutput=False)
    qb = nc.declare_dram_parameter("qb", [P, HPC], F32, isOutput=False)
    kb = nc.declare_dram_parameter("kb", [P, 1], F32, isOutput=False)
    vb = nc.declare_dram_parameter("vb", [P, 1], F32, isOutput=False)
    maskTb = nc.declare_dram_parameter("maskTb", [NT, T, 512], BF16,
                                       isOutput=False)
    cost = nc.declare_dram_parameter("cost", [P, T], BF16, isOutput=False)
    sint = nc.declare_dram_parameter("sint", [P, T], BF16, isOutput=False)
    rT = nc.declare_dram_parameter("rT", [P, P], BF16, isOutput=False)
    outTb = nc.declare_dram_parameter("outTb", [NT, JPC, 512], F32,
                                      isOutput=True)

    rs_in = nc.dram_tensor("rs_in", [NT, NCORE * JPC, 512], BF16)
    rs_out = nc.dram_tensor("rs_out", [NT, JPC, 512], BF16)

    with tile.TileContext(nc) as tc:
        with (
            tc.tile_pool(name="wpool", bufs=1) as wpool,
            tc.tile_pool(name="const", bufs=1) as constp,
            tc.tile_pool(name="qkv", bufs=1) as qkvp,
            tc.tile_pool(name="qc", bufs=2) as qcp,
            tc.tile_pool(name="aout", bufs=2) as aop,
            tc.tile_pool(name="ro", bufs=2) as rop,
            tc.tile_pool(name="xs", bufs=5) as xsp,
            tc.tile_pool(name="stage", bufs=4) as stagep,
            tc.tile_pool(name="att", bufs=4) as attp,
            tc.tile_pool(name="acc", bufs=4, space="PSUM") as accp,
            tc.tile_pool(name="satt", bufs=2, space="PSUM") as sattp,
            tc.tile_pool(name="attden", bufs=2, space="PSUM") as adp,
        ):
            # ---- resident weights / tables -------------------------------
            # wo split with tiny leading pieces so the first matmuls start
            # as soon as possible
            wo_s = wpool.tile([P, NKC, JPC], BF16)
            for klo, khi in ((0, 1), (1, 2), (2, 4), (4, 8), (8, 16), (16, 32)):
                nc.sync.dma_start(
                    out=wo_s[:, klo:khi, :],
                    in_=woT[klo * P:khi * P, :].rearrange("(k p) j -> p k j",
                                                          p=P))
            wk_s = wpool.tile([P, NKC, HD], BF16)
            nc.sync.dma_start(out=wk_s, in_=wkT[:, :].rearrange(
                "(k p) j -> p k j", p=P))
            wv_s = wpool.tile([P, NKC, HD], BF16)
            nc.sync.dma_start(out=wv_s, in_=wvT[:, :].rearrange(
                "(k p) j -> p k j", p=P))

            cos_s = constp.tile([P, T], BF16)
            sin_s = constp.tile([P, T], BF16)
            nc.sync.dma_start(out=cos_s, in_=cost[:, :])
            nc.sync.dma_start(out=sin_s, in_=sint[:, :])

            rT_s = constp.tile([P, P], BF16)
            nc.sync.dma_start(out=rT_s, in_=rT[:, :])
            qb_s = constp.tile([P, HPC], F32)
            kb_s = constp.tile([P, 1], F32)
            vb_s = constp.tile([P, 1], F32)
            nc.sync.dma_start(out=qb_s, in_=qb[:, :])
            nc.sync.dma_start(out=kb_s, in_=kb[:, :])
            nc.sync.dma_start(out=vb_s, in_=vb[:, :])

            # row-slice of wo for the contraction-sharded output projection
            wo3_s = wpool.tile([P, HPC, DIM], BF16)
            nc.sync.dma_start(out=wo3_s, in_=woR[:, :].rearrange(
                "(m p) o -> p m o", p=P))

            ones_s = constp.tile([P, P], BF16)
            nc.vector.memset(ones_s, 1.0)
            ident_s = constp.tile([P, P], BF16)
            make_identity(nc, ident_s)

            # persistent K/V in rope-d transposed layout (Q is per-chunk)
            kT_s = qkvp.tile([P, T], BF16)        # [hd, t]
            vN_s = qkvp.tile([P, TKC, HD], BF16)  # [tk%128, tk//128, hd]

            # ---- phases interleaved per t-chunk --------------------------
            # attention for chunk c only needs projections from chunks <= c,
            # so it is emitted right after chunk n=c's projections; its
            # partial output projection and ReduceScatter follow immediately
            # and overlap the next chunk's projections.
            for n in range(NT):
                ts = slice(n * 512, (n + 1) * 512)
                # x for this t-chunk: four 8-k-chunk quarter tiles (the
                # very first quarter arrives in two halves).  Issued on the
                # Pool queue BEFORE the previous chunk's psum-copy burst so
                # the data is in flight during that chunk's output proj.
                xtq = []
                for q in range(4):
                    ks = slice(q * 8 * P, (q + 1) * 8 * P)
                    xq = xsp.tile([P, 8, 512], BF16, name=f"xt{n}_{q}",
                                  tag="xs")
                    eng = nc.gpsimd
                    if n == 0 and q == 0:
                        for ha, hb in ((0, 2), (2, 8)):
                            eng.dma_start(
                                out=xq[:, ha:hb, :],
                                in_=xTb[n, ha * P:hb * P, :].rearrange(
                                    "(k p) t -> p k t", p=P))
                    else:
                        eng.dma_start(
                            out=xq,
                            in_=xTb[n, ks, :].rearrange("(k p) t -> p k t",
                                                        p=P))
                    xtq.append(xq)

                def xt_sl(k):
                    return xtq[k // 8][:, k % 8, :]

                acc_tiles = []
                for m in range(HPC + 2):  # 4 q-head tiles, k, v
                    pacc = accp.tile([P, 512], F32, name=f"pacc{n}_{m}",
                                     tag="acc")
                    acc_tiles.append(pacc)
                # q-pass
                for k in range(NKC):
                    for m in range(HPC):
                        nc.tensor.matmul(
                            acc_tiles[m],
                            lhsT=wo_s[:, k, m * P:(m + 1) * P],
                            rhs=xt_sl(k),
                            start=(k == 0),
                            stop=(k == NKC - 1),
                        )
                # q biases on Act while the kv-pass runs on PE
                braw_q = []
                for m in range(HPC):
                    braw = stagep.tile([P, 512], BF16, name=f"braw{n}_{m}",
                                       tag="braw")
                    nc.scalar.add(braw, acc_tiles[m], qb_s[:, m:m + 1])
                    braw_q.append(braw)
                # kv-pass
                for k in range(NKC):
                    nc.tensor.matmul(
                        acc_tiles[HPC], lhsT=wk_s[:, k, :], rhs=xt_sl(k),
                        start=(k == 0), stop=(k == NKC - 1),
                    )
                    nc.tensor.matmul(
                        acc_tiles[HPC + 1], lhsT=wv_s[:, k, :], rhs=xt_sl(k),
                        start=(k == 0), stop=(k == NKC - 1),
                    )
                # v bias first (its transpose is the earliest PE consumer
                # after the rotation matmuls), then k bias
                v_st = stagep.tile([P, 512], BF16, name=f"vst{n}", tag="braw")
                nc.scalar.add(v_st, acc_tiles[HPC + 1], vb_s[:, 0:1])
                brawk = stagep.tile([P, 512], BF16, name=f"brawk{n}",
                                    tag="braw")
                nc.scalar.add(brawk, acc_tiles[HPC], kb_s[:, 0:1])

                # rotation matmuls for q tiles + k tile (PE, after kv-pass)
                qc_s = qcp.tile([P, HPC, 512], BF16, name=f"qc{n}", tag="qc")
                rot_q = []
                for m in range(HPC):
                    rot_ps = accp.tile([P, 512], F32, name=f"rot{n}_{m}",
                                       tag="acc")
                    nc.tensor.matmul(rot_ps, lhsT=rT_s, rhs=braw_q[m],
                                     start=True, stop=True)
                    rot_q.append(rot_ps)
                rot_k = accp.tile([P, 512], F32, name=f"rotk{n}", tag="acc")
                nc.tensor.matmul(rot_k, lhsT=rT_s, rhs=brawk,
                                 start=True, stop=True)
                # v transpose into natural layout
                for j in range(4):
                    vt_ps = accp.tile([P, P], BF16, name=f"vt{n}_{j}",
                                      tag="acc")
                    nc.tensor.transpose(vt_ps, v_st[:, j * P:(j + 1) * P],
                                        ident_s)
                    nc.scalar.copy(vN_s[:, n * 4 + j, :], vt_ps)

                # rope combine on DVE (all-bf16 for 2x mode where possible)
                for m in range(HPC):
                    dst = qc_s[:, m, :]
                    tmp = stagep.tile([P, 512], BF16, name=f"tmp{n}_{m}",
                                      tag="stage")
                    nc.vector.tensor_mul(tmp, rot_q[m], sin_s[:, ts])
                    nc.vector.tensor_mul(dst, braw_q[m], cos_s[:, ts])
                    nc.vector.tensor_add(dst, dst, tmp)
                tmpk = stagep.tile([P, 512], BF16, name=f"tmpk{n}", tag="stage")
                nc.vector.tensor_mul(tmpk, rot_k, sin_s[:, ts])
                nc.vector.tensor_mul(kT_s[:, ts], brawk, cos_s[:, ts])
                nc.vector.tensor_add(kT_s[:, ts], kT_s[:, ts], tmpk)

                # ---- attention for chunk c = n ---------------------------
                c = n
                act_ks = [k for k in range(TKC) if klass[k, c] != MSK_SKIP]
                add_ks = [k for k in act_ks if klass[k, c] == MSK_ADD]
                mtiles = {}
                for k in add_ks:
                    mt = attp.tile([P, 512], BF16, name=f"mt{c}_{k}", tag="msk",
                                   bufs=max(2, len(add_ks) + 1))
                    nc.sync.dma_start(out=mt, in_=maskTb[c, k * P:(k + 1) * P, :])
                    mtiles[k] = mt

                attn_c = aop.tile([P, HPC, 512], BF16, name=f"ac{c}", tag="ac")
                nact = len(act_ks)
                pairs = [(h, i, k) for h in range(HPC)
                         for i, k in enumerate(act_ks)]
                state = {}

                def emit_score(h, i, k):
                    off = 0 if i == 0 else int(col0[k, c])
                    s_ps = sattp.tile([P, 512], F32, name=f"sps{c}_{h}_{k}",
                                      tag="satt")
                    nc.tensor.matmul(
                        s_ps[:, off:],
                        lhsT=kT_s[:, k * P:(k + 1) * P],
                        rhs=qc_s[:, h, off:],
                        start=True, stop=True,
                    )
                    if k in mtiles:
                        nc.vector.tensor_add(s_ps[:, off:], s_ps[:, off:],
                                             mtiles[k][:, off:])
                    e_sb = attp.tile([P, 512], BF16, name=f"e{c}_{h}_{k}",
                                     tag="exp", bufs=6)
                    # exp(SCALE * s + mask): mask was pre-divided by
                    # SCALE on the host, so the add can happen upstream.
                    nc.scalar.activation(
                        e_sb[:, off:], s_ps[:, off:],
                        mybir.ActivationFunctionType.Exp, scale=SCALE)
                    state[(h, i)] = (e_sb, off)

                def emit_avden(h, i):
                    if i == 0:
                        state[h, "apv"] = adp.tile(
                            [P, 512], F32, name=f"apv{c}_{h}", tag="attden")
                        state[h, "den"] = adp.tile(
                            [P, 512], F32, name=f"den{c}_{h}", tag="attden")
                    e_sb, off = state.pop((h, i))
                    k = act_ks[i]
                    nc.tensor.matmul(
                        state[h, "apv"][:, off:], lhsT=vN_s[:, k, :],
                        rhs=e_sb[:, off:],
                        start=(i == 0), stop=(i == nact - 1),
                    )
                    nc.tensor.matmul(
                        state[h, "den"][:, off:], lhsT=ones_s,
                        rhs=e_sb[:, off:],
                        start=(i == 0), stop=(i == nact - 1),
                    )
                    if i == nact - 1:
                        rcp = attp.tile([P, 512], F32, name=f"rcp{c}_{h}",
                                        tag="rcp", bufs=2)
                        nc.vector.reciprocal(rcp, state.pop((h, "den")))
                        nc.vector.tensor_mul(attn_c[:, h, :],
                                             state.pop((h, "apv")), rcp)

                # software-pipelined emission: the score for pair j+1 sits
                # between pair j's score and its exp-dependent matmuls, so
                # the PE never stalls the full exp latency.
                for j, (h, i, k) in enumerate(pairs):
                    emit_score(h, i, k)
                    if j >= 1:
                        ph, pi, _ = pairs[j - 1]
                        emit_avden(ph, pi)
                emit_avden(*pairs[-1][:2])

                # ---- phase 3: partial output projection + ReduceScatter --
                # contraction over this core's 512 attention dims, all 4096
                # output dims; psum banks borrowed from the (idle) attention
                # pools so the next chunk's projections keep their own.
                for jo in range(NKC):
                    po = (sattp if jo % 2 == 0 else adp).tile(
                        [P, 512], F32, name=f"po{n}_{jo}",
                        tag="satt" if jo % 2 == 0 else "attden")
                    for m in range(HPC):
                        nc.tensor.matmul(
                            po,
                            lhsT=wo3_s[:, m, jo * P:(jo + 1) * P],
                            rhs=attn_c[:, m, :],
                            start=(m == 0),
                            stop=(m == HPC - 1),
                        )
                    osb = attp.tile([P, 512], BF16, name=f"osb{n}_{jo}",
                                    tag="osb", bufs=4)
                    # GPSIMD cannot read PSUM; alternate DVE/Act for the
                    # psum->sbuf downcast copies
                    if jo % 2 == 0:
                        nc.vector.tensor_copy(osb, po)
                    else:
                        nc.scalar.copy(osb, po)
                    nc.sync.dma_start(out=rs_in[n, jo * P:(jo + 1) * P, :],
                                      in_=osb)
                nc.gpsimd.collective_compute(
                    "ReduceScatter",
                    mybir.AluOpType.add,
                    replica_groups=[list(range(NCORE))],
                    ins=[rs_in[n]],
                    outs=[rs_out[n]],
                )

                # ---- phase 4: bias + writeback (SP + DVE) ----------------
                ro_s = rop.tile([P, HPC, 512], BF16, name=f"ro{n}", tag="ro")
                nc.sync.dma_start(out=ro_s, in_=rs_out[n].rearrange(
                    "(m p) t -> p m t", p=P))
                for m in range(HPC):
                    o_sb = stagep.tile([P, 512], F32, name=f"o{n}_{m}",
                                       tag="ostage", bufs=2)
                    nc.vector.tensor_scalar_add(o_sb, ro_s[:, m, :],
                                                qb_s[:, m:m + 1])
                    nc.sync.dma_start(out=outTb[n, m * P:(m + 1) * P, :],
                                      in_=o_sb)

    legalize_waits(nc)
    return nc


def _marshal_inputs(x, freqs_cos, freqs_sin, mask, wk_w, wk_b, wv_w, wv_b,
                    wo_w, wo_b):
    bf = ml_dtypes.bfloat16
    x = np.asarray(x, np.float32)
    mask = np.asarray(mask, np.float32)
    cos = np.asarray(freqs_cos, np.float32)
    sin = np.asarray(freqs_sin, np.float32)
    wk_w = np.asarray(wk_w, np.float32)
    wv_w = np.asarray(wv_w, np.float32)
    wo_w = np.asarray(wo_w, np.float32)
    wk_b = np.asarray(wk_b, np.float32)
    wv_b = np.asarray(wv_b, np.float32)
    wo_b = np.asarray(wo_b, np.float32)

    xT = x.reshape(T, DIM).T                       # (DIM, T)
    xTb = np.ascontiguousarray(
        xT.reshape(DIM, NT, 512).transpose(1, 0, 2).astype(bf))
    # mask applied on-device as exp(SCALE*s + SCALE*maskT): pre-divide, and
    # reblock (tq-chunk, tk, tq') so every mask tile DMA is contiguous
    maskT = mask.T / np.float32(SCALE)             # (tk, tq)
    maskTb = np.ascontiguousarray(
        maskT.reshape(T, NT, 512).transpose(1, 0, 2).astype(bf))

    cos2 = np.repeat(cos.T, 2, axis=0)  # (128, T): rows 2i,2i+1 = cos[:, i]
    sin2 = np.repeat(sin.T, 2, axis=0)

    # rotation matmul constant: out = R @ q with rot[2i] = -q[2i+1],
    # rot[2i+1] = q[2i]; lhsT layout (R transposed).
    RT = np.zeros((P, P), np.float32)
    idx = np.arange(0, P, 2)
    RT[idx + 1, idx] = -1.0
    RT[idx, idx + 1] = 1.0

    common = dict(
        xTb=xTb, maskTb=maskTb,
        cost=np.ascontiguousarray(cos2.astype(bf)),
        sint=np.ascontiguousarray(sin2.astype(bf)),
        rT=RT.astype(bf),
    )

    woT_full = wo_w.T  # (DIM in, DIM out): woT_full[d, o] = wo_w[o, d]
    in_maps = []
    for cix in range(NCORE):
        jlo = cix * JPC
        klo = cix * HD
        m = dict(common)
        m["woT"] = np.ascontiguousarray(wo_w[jlo:jlo + JPC, :].T.astype(bf))
        m["woR"] = np.ascontiguousarray(woT_full[jlo:jlo + JPC, :].astype(bf))
        m["wkT"] = np.ascontiguousarray(wk_w[klo:klo + HD, :].T.astype(bf))
        m["wvT"] = np.ascontiguousarray(wv_w[klo:klo + HD, :].T.astype(bf))
        m["qb"] = np.ascontiguousarray(wo_b[jlo:jlo + JPC].reshape(HPC, P).T)
        m["kb"] = np.ascontiguousarray(wk_b[klo:klo + HD].reshape(1, P).T)
        m["vb"] = np.ascontiguousarray(wv_b[klo:klo + HD].reshape(1, P).T)
        in_maps.append(m)
    return in_maps, mask


def run(inputs, trace=False):
    """Build, run on 8 cores, return (full_output, BassKernelResults)."""
    in_maps, mask = _marshal_inputs(
        inputs["x"], inputs["freqs_cos"], inputs["freqs_sin"], inputs["mask"],
        inputs["wk_w"], inputs["wk_b"], inputs["wv_w"], inputs["wv_b"],
        inputs["wo_w"], inputs["wo_b"])
    klass, col0 = _classify_mask(mask)
    nc = _build_module(klass, col0)
    res = run_bass_kernel_spmd(nc, in_maps, core_ids=list(range(NCORE)),
                               trace=trace)
    out = np.empty((DIM, T), np.float32)
    for cix in range(NCORE):
        ob = res.results[cix]["outTb"]          # (NT, JPC, 512)
        for n in range(NT):
            out[cix * JPC:(cix + 1) * JPC, n * 512:(n + 1) * 512] = ob[n]
    out = out.T  # (T, DIM)
    return np.ascontiguousarray(out[None, :, :]).astype(np.float32), res


def kernel(**inputs):
    out, _ = run(inputs, trace=False)
    return out
